# revision 5
# baseline (speedup 1.0000x reference)
"""Distributed Trainium2 attention kernel (8 NeuronCores).

Problem: multi-head attention, B=2, S=2048, D_MODEL=768, 12 heads x 64.
Sharding: batch (2) x head-groups (4 groups of 3 heads) = 8 cores.
Each core computes a disjoint [2048, 192] block of the output -> no
collectives; the host slices/assembles.

Per-core layout strategy (TensorE contracts over the partition dim):
  - host supplies q/k/v pre-transposed as qT/kT/vT [768, 2048] (layout
    marshaling only; all FLOPs stay on device)
  - projections (bf16): qwT/kwT produced transposed [d, s] (lhsT = W
    column block, rhs = xT), vw produced natural [s, d] (lhsT = vT tile,
    rhs = Wv) with a ones-column appended for softmax row-sums
  - scores computed transposed [k, q] so exp output feeds A@V directly
    as lhsT; v_mask enters as ACT's per-partition bias; 1/sqrt(dk) as
    ACT's scale; row-sum normalization + q_mask as per-partition scalar
    multiply after A@V.
"""

import sys
import types

import numpy as np

if "/opt/trn_rl_repo" not in sys.path:
    sys.path.insert(0, "/opt/trn_rl_repo")

B = 2
S = 2048
D = 768
HEADS = 12
DK = 64
GROUPS = 4          # head-groups (tensor parallel)
HPG = 3             # heads per group
GD = HPG * DK       # 192 output cols per core
NEG_BIG = 1.0e10
SCALE = 0.125       # 1/sqrt(64)
N_CORES = 8
ST = S // 128       # 16 sequence tiles
CC = D // 128       # 6 contraction chunks


def _install_ntff_hook():
    """bass_utils wants antenv.axon_hooks (absent in this image). Shim it and
    register the ctypes-based NTFF profile hook from the boot package."""
    if "antenv.axon_hooks" in sys.modules:
        return
    try:
        import antenv
        from trn_agent_boot.trn_boot import _ntff_profile_via_ctypes

        mod = types.ModuleType("antenv.axon_hooks")
        _state = {"hook": _ntff_profile_via_ctypes("/opt/axon/libaxon_pjrt.so")}
        mod.get_axon_ntff_profile_hook = lambda: _state["hook"]
        mod.set_axon_ntff_profile_hook = lambda h: _state.__setitem__("hook", h)
        sys.modules["antenv.axon_hooks"] = mod
        antenv.axon_hooks = mod
    except Exception:
        pass


_NC_CACHE = {}


def _build():
    if "nc" in _NC_CACHE:
        return _NC_CACHE["nc"]

    from concourse import bacc, mybir
    from concourse.tile import TileContext

    F32 = mybir.dt.float32
    BF16 = mybir.dt.bfloat16
    Exp = mybir.ActivationFunctionType.Exp

    nc = bacc.Bacc()

    qT = nc.declare_dram_parameter("qT", [D, S], F32, isOutput=False)
    kT = nc.declare_dram_parameter("kT", [D, S], F32, isOutput=False)
    vT = nc.declare_dram_parameter("vT", [D, S], F32, isOutput=False)
    wq = nc.declare_dram_parameter("wq", [D, GD], F32, isOutput=False)
    wk = nc.declare_dram_parameter("wk", [D, GD], F32, isOutput=False)
    wv = nc.declare_dram_parameter("wv", [D, GD], F32, isOutput=False)
    vmask_t = nc.declare_dram_parameter("vmask_t", [128, ST], F32, isOutput=False)
    qmask_t = nc.declare_dram_parameter("qmask_t", [128, ST], F32, isOutput=False)
    out = nc.declare_dram_parameter("out", [S, GD], F32, isOutput=True)

    with TileContext(nc) as tc:
        with (
            tc.tile_pool(name="persist", bufs=1) as persist,
            tc.tile_pool(name="stage", bufs=3) as stage,
            tc.tile_pool(name="probs", bufs=3) as probsp,
            tc.tile_pool(name="small", bufs=8) as small,
            tc.tile_pool(name="psum", bufs=1, space="PSUM") as psum,
        ):
            # ---- weights: DMA f32, cast to bf16 [128, CC, GD] ----
            w_b = {}
            for name, par in (("wq", wq), ("wk", wk), ("wv", wv)):
                ws = stage.tile([128, CC, GD], F32, tag="wstage")
                nc.sync.dma_start(out=ws[:], in_=par[:].rearrange("(o p) f -> p o f", p=128))
                wb = persist.tile([128, CC, GD], BF16, tag=f"{name}_b", name=f"{name}_b")
                nc.vector.tensor_copy(out=wb[:], in_=ws[:])
                w_b[name] = wb

            # ---- masks ----
            vm = small.tile([128, ST], F32, tag="vmstage")
            nc.sync.dma_start(out=vm[:], in_=vmask_t[:])
            vbias = persist.tile([128, ST], F32, tag="vbias")
            # (vm * NEG_BIG) - NEG_BIG  -> 0 where mask==1, -1e10 where 0
            nc.vector.tensor_scalar(
                vbias[:], vm[:], NEG_BIG, -NEG_BIG,
                mybir.AluOpType.mult, mybir.AluOpType.add,
            )
            qm = persist.tile([128, ST], F32, tag="qmask")
            nc.sync.dma_start(out=qm[:], in_=qmask_t[:])

            # ---- activations: DMA f32 chunks, cast to bf16 [128, CC, S] ----
            xb = {}
            for name, par in (("qT", qT), ("kT", kT), ("vT", vT)):
                xb[name] = persist.tile(
                    [128, CC, S], BF16, tag=f"{name}_b", name=f"{name}_b"
                )
            for c in range(CC):
                for name, par in (("qT", qT), ("kT", kT), ("vT", vT)):
                    st_ = stage.tile([128, S], F32, tag="xstage")
                    nc.sync.dma_start(out=st_[:], in_=par[c * 128:(c + 1) * 128, :])
                    nc.vector.tensor_copy(out=xb[name][:, c, :], in_=st_[:])

            # ---- projections: qwT / kwT transposed [d, s] (pair + single) ----
            # pair: heads 0,1 -> [128, S]; single: head 2 -> [64, S]
            proj = {}
            for name, wname in (("q", "wq"), ("k", "wk")):
                pair = persist.tile([128, S], BF16, tag=f"{name}w01")
                single = persist.tile([64, S], BF16, tag=f"{name}w2")
                for dlo, dhi, dst in ((0, 128, pair), (128, GD, single)):
                    m = dhi - dlo
                    ps = psum.tile([128, S], F32, tag="scores")
                    for c in range(CC):
                        for qb in range(4):
                            nc.tensor.matmul(
                                ps[:m, qb * 512:(qb + 1) * 512],
                                w_b[wname][:, c, dlo:dhi],
                                xb[f"{name}T"][:, c, qb * 512:(qb + 1) * 512],
                                start=(c == 0),
                                stop=(c == CC - 1),
                            )
                    nc.vector.tensor_copy(out=dst[:], in_=ps[:m, :])
                proj[name] = (pair, single)

            def head_qw(h):
                pair, single = proj["q"]
                return single[0:64, :] if h == 2 else pair[h * 64:(h + 1) * 64, :]

            def head_kw(h):
                pair, single = proj["k"]
                return single[0:64, :] if h == 2 else pair[h * 64:(h + 1) * 64, :]

            # ---- projection: vw natural [s, dv] with ones column ----
            vw = []
            for h in range(HPG):
                t = persist.tile([128, ST, DK + 1], BF16, tag=f"vw{h}")
                nc.vector.memset(t[:, :, DK], 1.0)
                vw.append(t)
            for st_i in range(ST):
                ps = psum.tile([128, S], F32, tag="accs")
                for c in range(CC):
                    nc.tensor.matmul(
                        ps[:, 0:GD],
                        xb["vT"][:, c, st_i * 128:(st_i + 1) * 128],
                        w_b["wv"][:, c, :],
                        start=(c == 0),
                        stop=(c == CC - 1),
                    )
                for h in range(HPG):
                    nc.vector.tensor_copy(
                        out=vw[h][:, st_i, 0:DK], in_=ps[:, h * DK:(h + 1) * DK]
                    )

            # ---- attention ----
            outsb = persist.tile([128, ST, GD], F32, tag="outsb")
            for h in range(HPG):
                qw_h = head_qw(h)
                kw_h = head_kw(h)
                accs = psum.tile([128, S], F32, tag="accs")
                # 4 accumulators share each PSUM bank, so per-slice start=True
                # would clear siblings' has-written state (2KB zero-region
                # granularity). Zero the banks once, then always accumulate.
                nc.vector.memset(accs[:], 0.0)
                for kt in range(ST):
                    sc = psum.tile([128, S], F32, tag="scores")
                    lhsT = kw_h[:, kt * 128:(kt + 1) * 128]
                    for qb in range(4):
                        nc.tensor.matmul(
                            sc[:, qb * 512:(qb + 1) * 512],
                            lhsT,
                            qw_h[:, qb * 512:(qb + 1) * 512],
                            start=True,
                            stop=True,
                        )
                    probs = probsp.tile([128, S], BF16, tag="probs")
                    nc.scalar.activation(
                        probs[:], sc[:], Exp,
                        bias=vbias[:, kt:kt + 1], scale=SCALE,
                    )
                    for qt in range(ST):
                        nc.tensor.matmul(
                            accs[:, qt * 128:qt * 128 + DK + 1],
                            probs[:, qt * 128:(qt + 1) * 128],
                            vw[h][:, kt, :],
                            start=False,
                            stop=(kt == ST - 1),
                            skip_group_check=True,
                        )
                for qt in range(ST):
                    off = qt * 128
                    inv = small.tile([128, 1], F32, tag="inv")
                    nc.vector.reciprocal(inv[:], accs[:, off + DK:off + DK + 1])
                    scl = small.tile([128, 1], F32, tag="scl")
                    nc.vector.tensor_mul(out=scl[:], in0=inv[:], in1=qm[:, qt:qt + 1])
                    nc.vector.tensor_scalar_mul(
                        outsb[:, qt, h * DK:(h + 1) * DK],
                        accs[:, off:off + DK],
                        scl[:],
                    )

            nc.sync.dma_start(
                out=out[:].rearrange("(t p) d -> p t d", p=128),
                in_=outsb[:],
            )

    nc.finalize()
    _NC_CACHE["nc"] = nc
    return nc


def _shard_inputs(q, k, v, v_mask, q_mask, Wq, Wk, Wv):
    """Build per-core in_maps. Core i = (batch i//4, head-group i%4)."""
    qT = [np.ascontiguousarray(q[b].T) for b in range(B)]
    kTt = [np.ascontiguousarray(k[b].T) for b in range(B)]
    vTt = [np.ascontiguousarray(v[b].T) for b in range(B)]
    vm = [np.ascontiguousarray(v_mask[b].reshape(ST, 128).T) for b in range(B)]
    qm = [np.ascontiguousarray(q_mask[b].reshape(ST, 128).T) for b in range(B)]
    wqs = [np.ascontiguousarray(Wq[:, g * GD:(g + 1) * GD]) for g in range(GROUPS)]
    wks = [np.ascontiguousarray(Wk[:, g * GD:(g + 1) * GD]) for g in range(GROUPS)]
    wvs = [np.ascontiguousarray(Wv[:, g * GD:(g + 1) * GD]) for g in range(GROUPS)]
    in_maps = []
    for i in range(N_CORES):
        b, g = divmod(i, GROUPS)
        in_maps.append({
            "qT": qT[b], "kT": kTt[b], "vT": vTt[b],
            "wq": wqs[g], "wk": wks[g], "wv": wvs[g],
            "vmask_t": vm[b], "qmask_t": qm[b],
        })
    return in_maps


def run(inputs, trace=False):
    """Run the SPMD kernel. Returns (full_output [B,S,768] f32, exec_time_ns)."""
    _install_ntff_hook()
    from concourse.bass_utils import run_bass_kernel_spmd

    nc = _build()
    in_maps = _shard_inputs(**{k: np.asarray(val) for k, val in inputs.items()})
    res = run_bass_kernel_spmd(
        nc, in_maps, core_ids=list(range(N_CORES)), trace=trace
    )
    o = np.empty((B, S, HEADS * DK), dtype=np.float32)
    for i in range(N_CORES):
        b, g = divmod(i, GROUPS)
        o[b, :, g * GD:(g + 1) * GD] = res.results[i]["out"]
    return o, res.exec_time_ns


def kernel(**inputs) -> np.ndarray:
    o, _ = run(inputs, trace=False)
    return o


if __name__ == "__main__":
    rng = np.random.default_rng(0)
    ins = {
        "q": rng.standard_normal((B, S, D), dtype=np.float32),
        "k": rng.standard_normal((B, S, D), dtype=np.float32),
        "v": rng.standard_normal((B, S, D), dtype=np.float32),
        "v_mask": np.ones((B, S), np.float32),
        "q_mask": np.ones((B, S), np.float32),
        "Wq": (rng.standard_normal((D, 768)) / np.sqrt(D)).astype(np.float32),
        "Wk": (rng.standard_normal((D, 768)) / np.sqrt(D)).astype(np.float32),
        "Wv": (rng.standard_normal((D, 768)) / np.sqrt(D)).astype(np.float32),
    }
    o, t = run(ins, trace=True)
    print("exec_time_ns:", t, "out[0,0,:4]:", o[0, 0, :4])


# revision 6
# speedup vs baseline: 1.8110x; 1.8110x over previous
"""Distributed Trainium2 attention kernel (8 NeuronCores).

Problem: multi-head attention, B=2, S=2048, D_MODEL=768, 12 heads x 64.
Sharding: batch (2) x head-groups (4 groups of 3 heads) = 8 cores.
Each core computes a disjoint [2048, 192] block of the output -> no
collectives; the host slices/assembles.

Per-core pipeline (TensorE contracts over the partition dim):
  - host supplies q/k/v pre-transposed as qT/kT/vT [768, 2048] (layout
    marshaling only; all FLOPs stay on device)
  - inputs DMA'd f32->bf16 via gpsimd cast-DMA (no staging copies)
  - projections (bf16): qwT/kwT produced transposed [d, s], vw natural
    [s, d] with a ones-column appended for softmax row-sums
  - scores computed transposed [k, q] in [128, 1024] PSUM tiles
    (double-buffered, 2 banks each) so exp output feeds A@V directly as
    lhsT; v_mask enters as ACT's per-partition bias; 1/sqrt(dk) as ACT's
    scale
  - A@V accumulators: 8 x [128, 65] packed in a [128, 1024] PSUM tile
    (2 banks, double-buffered). PSUM start=True clears has-written for a
    whole 2KB zero-region, so shared-bank accumulators use memset +
    start=False accumulation instead.
  - row-sum normalization + q_mask: per-partition scalar ops on DVE.
"""

import sys
import types

import numpy as np

if "/opt/trn_rl_repo" not in sys.path:
    sys.path.insert(0, "/opt/trn_rl_repo")

B = 2
S = 2048
D = 768
HEADS = 12
DK = 64
GROUPS = 4          # head-groups (tensor parallel)
HPG = 3             # heads per group
GD = HPG * DK       # 192 output cols per core
NEG_BIG = 1.0e10
SCALE = 0.125       # 1/sqrt(64)
N_CORES = 8
ST = S // 128       # 16 sequence tiles
CC = D // 128       # 6 contraction chunks
QH = 1024           # q-half width for score tiles
QT_H = QH // 128    # 8 q-tiles per half


def _install_ntff_hook():
    """bass_utils wants antenv.axon_hooks (absent in this image). Shim it and
    register the ctypes-based NTFF profile hook from the boot package."""
    if "antenv.axon_hooks" in sys.modules:
        return
    try:
        import antenv
        from trn_agent_boot.trn_boot import _ntff_profile_via_ctypes

        mod = types.ModuleType("antenv.axon_hooks")
        _state = {"hook": _ntff_profile_via_ctypes("/opt/axon/libaxon_pjrt.so")}
        mod.get_axon_ntff_profile_hook = lambda: _state["hook"]
        mod.set_axon_ntff_profile_hook = lambda h: _state.__setitem__("hook", h)
        sys.modules["antenv.axon_hooks"] = mod
        antenv.axon_hooks = mod
    except Exception:
        pass


_NC_CACHE = {}


def _build():
    if "nc" in _NC_CACHE:
        return _NC_CACHE["nc"]

    from concourse import bacc, mybir
    from concourse.tile import TileContext

    F32 = mybir.dt.float32
    BF16 = mybir.dt.bfloat16
    Exp = mybir.ActivationFunctionType.Exp

    nc = bacc.Bacc(num_swdge_queues=4)

    qT = nc.declare_dram_parameter("qT", [D, S], F32, isOutput=False)
    kT = nc.declare_dram_parameter("kT", [D, S], F32, isOutput=False)
    vT = nc.declare_dram_parameter("vT", [D, S], F32, isOutput=False)
    wq = nc.declare_dram_parameter("wq", [D, GD], F32, isOutput=False)
    wk = nc.declare_dram_parameter("wk", [D, GD], F32, isOutput=False)
    wv = nc.declare_dram_parameter("wv", [D, GD], F32, isOutput=False)
    vmask_t = nc.declare_dram_parameter("vmask_t", [128, ST], F32, isOutput=False)
    qmask_t = nc.declare_dram_parameter("qmask_t", [128, ST], F32, isOutput=False)
    out = nc.declare_dram_parameter("out", [S, GD], F32, isOutput=True)

    with TileContext(nc) as tc:
        with (
            tc.tile_pool(name="persist", bufs=1) as persist,
            tc.tile_pool(name="probs", bufs=4) as probsp,
            tc.tile_pool(name="small", bufs=8) as small,
            tc.tile_pool(name="psum", bufs=2, space="PSUM") as psum,
        ):
            # ---- weights + masks (cast-DMA f32 -> bf16) ----
            w_b = {}
            for name, par in (("wq", wq), ("wk", wk), ("wv", wv)):
                wb = persist.tile([128, CC, GD], BF16, tag=f"{name}_b", name=f"{name}_b")
                nc.gpsimd.dma_start(out=wb[:], in_=par[:].rearrange("(o p) f -> p o f", p=128))
                w_b[name] = wb
            vm = small.tile([128, ST], F32, tag="vmstage")
            nc.sync.dma_start(out=vm[:], in_=vmask_t[:])
            vbias = persist.tile([128, ST], F32, tag="vbias")
            # (vm * NEG_BIG) - NEG_BIG  -> 0 where mask==1, -1e10 where 0
            nc.vector.tensor_scalar(
                vbias[:], vm[:], NEG_BIG, -NEG_BIG,
                mybir.AluOpType.mult, mybir.AluOpType.add,
            )
            qm = persist.tile([128, ST], F32, tag="qmask")
            nc.sync.dma_start(out=qm[:], in_=qmask_t[:])

            # ---- activations: gpsimd cast-DMA per c-chunk (q,k first, v last) ----
            xb = {}
            for name in ("qT", "kT", "vT"):
                xb[name] = persist.tile(
                    [128, CC, S], BF16, tag=f"{name}_b", name=f"{name}_b"
                )
            for c in range(CC):
                for name, par in (("qT", qT), ("kT", kT)):
                    nc.gpsimd.dma_start(
                        out=xb[name][:, c, :], in_=par[c * 128:(c + 1) * 128, :]
                    )
            for c in range(CC):
                nc.gpsimd.dma_start(
                    out=xb["vT"][:, c, :], in_=vT[c * 128:(c + 1) * 128, :]
                )

            # ---- projections: qwT / kwT transposed [d, s] (pair + single) ----
            proj = {}
            for name, wname in (("q", "wq"), ("k", "wk")):
                pair = persist.tile([128, S], BF16, tag=f"{name}w01", name=f"{name}w01")
                single = persist.tile([64, S], BF16, tag=f"{name}w2", name=f"{name}w2")
                for dlo, dhi, dst in ((0, 128, pair), (128, GD, single)):
                    m = dhi - dlo
                    for sh in range(2):
                        ps = psum.tile([128, QH], F32, tag="scores")
                        for c in range(CC):
                            for qb in range(2):
                                nc.tensor.matmul(
                                    ps[:m, qb * 512:(qb + 1) * 512],
                                    w_b[wname][:, c, dlo:dhi],
                                    xb[f"{name}T"][:, c, sh * QH + qb * 512:sh * QH + (qb + 1) * 512],
                                    start=(c == 0),
                                    stop=(c == CC - 1),
                                )
                        nc.vector.tensor_copy(
                            out=dst[:, sh * QH:(sh + 1) * QH], in_=ps[:m, :]
                        )
                proj[name] = (pair, single)

            def head_slice(t, h):
                pair, single = t
                return single[0:64, :] if h == 2 else pair[h * 64:(h + 1) * 64, :]

            # ---- projection: vw natural [s, dv] with ones column ----
            vw = []
            for h in range(HPG):
                t = persist.tile([128, ST, DK + 1], BF16, tag=f"vw{h}", name=f"vw{h}")
                nc.vector.memset(t[:, :, DK], 1.0)
                vw.append(t)
            for st_i in range(ST):
                ps = psum.tile([128, QH], F32, tag="accs")
                for c in range(CC):
                    nc.tensor.matmul(
                        ps[:, 0:GD],
                        xb["vT"][:, c, st_i * 128:(st_i + 1) * 128],
                        w_b["wv"][:, c, :],
                        start=(c == 0),
                        stop=(c == CC - 1),
                    )
                for h in range(HPG):
                    nc.vector.tensor_copy(
                        out=vw[h][:, st_i, 0:DK], in_=ps[:, h * DK:(h + 1) * DK]
                    )

            # ---- attention ----
            outsb = persist.tile([128, ST, GD], F32, tag="outsb")
            for h in range(HPG):
                qw_h = head_slice(proj["q"], h)
                kw_h = head_slice(proj["k"], h)
                for qh in range(2):
                    accs = psum.tile([128, QH], F32, tag="accs")
                    nc.vector.memset(accs[:], 0.0)
                    for kt in range(ST):
                        sc = psum.tile([128, QH], F32, tag="scores")
                        lhsT = kw_h[:, kt * 128:(kt + 1) * 128]
                        for qb in range(2):
                            nc.tensor.matmul(
                                sc[:, qb * 512:(qb + 1) * 512],
                                lhsT,
                                qw_h[:, qh * QH + qb * 512:qh * QH + (qb + 1) * 512],
                                start=True,
                                stop=True,
                            )
                        probs = probsp.tile([128, QH], BF16, tag="probs")
                        nc.scalar.activation(
                            probs[:], sc[:], Exp,
                            bias=vbias[:, kt:kt + 1], scale=SCALE,
                        )
                        for qt in range(QT_H):
                            nc.tensor.matmul(
                                accs[:, qt * 128:qt * 128 + DK + 1],
                                probs[:, qt * 128:(qt + 1) * 128],
                                vw[h][:, kt, :],
                                start=False,
                                stop=(kt == ST - 1),
                                skip_group_check=True,
                            )
                    for qt in range(QT_H):
                        qt_g = qh * QT_H + qt
                        off = qt * 128
                        inv = small.tile([128, 1], F32, tag="inv")
                        nc.vector.reciprocal(inv[:], accs[:, off + DK:off + DK + 1])
                        scl = small.tile([128, 1], F32, tag="scl")
                        nc.vector.tensor_mul(
                            out=scl[:], in0=inv[:], in1=qm[:, qt_g:qt_g + 1]
                        )
                        nc.vector.tensor_scalar_mul(
                            outsb[:, qt_g, h * DK:(h + 1) * DK],
                            accs[:, off:off + DK],
                            scl[:],
                        )

            nc.sync.dma_start(
                out=out[:].rearrange("(t p) d -> p t d", p=128),
                in_=outsb[:],
            )

    nc.finalize()
    _NC_CACHE["nc"] = nc
    return nc


def _shard_inputs(q, k, v, v_mask, q_mask, Wq, Wk, Wv):
    """Build per-core in_maps. Core i = (batch i//4, head-group i%4)."""
    qT = [np.ascontiguousarray(q[b].T) for b in range(B)]
    kTt = [np.ascontiguousarray(k[b].T) for b in range(B)]
    vTt = [np.ascontiguousarray(v[b].T) for b in range(B)]
    vm = [np.ascontiguousarray(v_mask[b].reshape(ST, 128).T) for b in range(B)]
    qm = [np.ascontiguousarray(q_mask[b].reshape(ST, 128).T) for b in range(B)]
    wqs = [np.ascontiguousarray(Wq[:, g * GD:(g + 1) * GD]) for g in range(GROUPS)]
    wks = [np.ascontiguousarray(Wk[:, g * GD:(g + 1) * GD]) for g in range(GROUPS)]
    wvs = [np.ascontiguousarray(Wv[:, g * GD:(g + 1) * GD]) for g in range(GROUPS)]
    in_maps = []
    for i in range(N_CORES):
        b, g = divmod(i, GROUPS)
        in_maps.append({
            "qT": qT[b], "kT": kTt[b], "vT": vTt[b],
            "wq": wqs[g], "wk": wks[g], "wv": wvs[g],
            "vmask_t": vm[b], "qmask_t": qm[b],
        })
    return in_maps


def run(inputs, trace=False):
    """Run the SPMD kernel. Returns (full_output [B,S,768] f32, exec_time_ns)."""
    _install_ntff_hook()
    from concourse.bass_utils import run_bass_kernel_spmd

    nc = _build()
    in_maps = _shard_inputs(**{k: np.asarray(val) for k, val in inputs.items()})
    res = run_bass_kernel_spmd(
        nc, in_maps, core_ids=list(range(N_CORES)), trace=trace
    )
    o = np.empty((B, S, HEADS * DK), dtype=np.float32)
    for i in range(N_CORES):
        b, g = divmod(i, GROUPS)
        o[b, :, g * GD:(g + 1) * GD] = res.results[i]["out"]
    return o, res.exec_time_ns


def kernel(**inputs) -> np.ndarray:
    o, _ = run(inputs, trace=False)
    return o


if __name__ == "__main__":
    rng = np.random.default_rng(0)
    ins = {
        "q": rng.standard_normal((B, S, D), dtype=np.float32),
        "k": rng.standard_normal((B, S, D), dtype=np.float32),
        "v": rng.standard_normal((B, S, D), dtype=np.float32),
        "v_mask": np.ones((B, S), np.float32),
        "q_mask": np.ones((B, S), np.float32),
        "Wq": (rng.standard_normal((D, 768)) / np.sqrt(D)).astype(np.float32),
        "Wk": (rng.standard_normal((D, 768)) / np.sqrt(D)).astype(np.float32),
        "Wv": (rng.standard_normal((D, 768)) / np.sqrt(D)).astype(np.float32),
    }
    o, t = run(ins, trace=True)
    print("exec_time_ns:", t, "out[0,0,:4]:", o[0, 0, :4])
